# revision 73
# baseline (speedup 1.0000x reference)
"""Trainium2 Bass kernel for nn_DMPNN_Change_678604832935 (8-core SPMD DMPNN+Set2Set).

Sharding: each core owns 64 consecutive graphs (batch is sorted) plus all edges
whose dst node falls in those graphs — so segment_sum is core-local and no
collectives are needed.  The node-side h0 = relu(x@W0+b0) is recomputed per
edge-source from host-gathered x rows; since segment_sum is linear and sits
between the Wm2 matmul and the root update, m@Wm2 is folded to the node side
(16x fewer FLOPs), with deg(n)*bm2 as a rank-1 correction.

Perf structure (PE-throughput-bound edge phase; ~0.39 ms/core in TimelineSim
vs 0.49 for the pre-optimization version; edge-phase PE idle is ~5us):
- Per-core graphs are sorted by edge count so grid position j holds rank-j
  graphs on every core; per-position edge-chunk counts (max over cores) then
  track rank statistics -> near-minimal edge padding with ONE SPMD program.
- Per-position slot windows (32-aligned max graph size at that rank) are
  bin-packed into 128-slot grid bins (PE out-base rules: w<=64 at base 0/64,
  else base 0), shrinking the scatter stream and the attention loop (44 bins
  vs 64 graphs).
- The edge loop is a deep software pipeline over 2-chunk PAIRS, built
  around the fact that each engine executes its queue IN ORDER: at pair p the
  PE runs eaw(p) (one [128,512] psum, 3 matmuls/chunk), the h0 recompute for
  pair p+4, the scatter for pair p-3, root updates for graphs that finished
  at p-4 — so no PE instruction ever waits on a relu issued fewer than 3
  iterations earlier (lookahead depth was tuned by TimelineSim: 2->3 on the
  tp stage alone was worth 19us).  m1 relu is one [128,512] ACT op per pair; the
  per-graph agg drain + out-relu go to DVE (ACT~1010 / DVE~930 /
  PE~1100 ns per pair).  PSUM: tp ring 2 banks + eaw ring 3 + agg ring 2
  + po 1 = 8 banks exactly.
- P1 (grid h0) is interleaved one 256-col tile per other pair, emission
  ordered so each tile lands before the first root update that reads it.
- Startup: weights blob + xTg DMA first; s_out's zero+valid fill arrives as
  a DMA'd constant (no 12us DVE memset); 3-deep strip prefetch.
- Set2Set: with bl == 0 (checked at prep), step 0 collapses to a uniform
  average, computed at P3 entry as one accumulating matmul sweep over the
  44 bins (frees a P2 psum bank and 6.8us of P2-phase PE time).  The
  attention mask is folded into exp via a log-mask (gw = exp(Gln + e), f16
  flushes masked terms to 0), pipelining T->V->A->T per bin with no barrier.
  Softmax uses unnormalized exp (|e|<~8, validated); sigmoid is synthesized
  from tanh (one ACT table set); i/f gates share one [64,512] tanh.
- GPSIMD is unusable here: no PSUM port, and TensorScalarPtr fails the
  real ISA check on Pool (walrus rejects it; CoreSim does not).
"""

import os
import sys

for _p in ("/opt/trn_rl_repo", "/root/.axon_site/_ro/trn_rl_repo"):
    if os.path.isdir(_p) and _p not in sys.path:
        sys.path.append(_p)

import numpy as np

import concourse.bass as bass
import concourse.bacc as bacc
import concourse.mybir as mybir
import concourse.tile as tile
from concourse.bass_utils import run_bass_kernel_spmd

F16 = mybir.dt.float16
F32 = mybir.dt.float32
AF = mybir.ActivationFunctionType
ALU = mybir.AluOpType

N_NODES = 30000
FIN = 25
FE = 14
D = 256
N_GRAPHS = 512
N_CORES = 8
GPC = N_GRAPHS // N_CORES      # graphs per core
SLOT = 128                     # grid slots per graph
GRID = GPC * SLOT              # grid rows per core
NCH = GRID // 128              # grid chunks per core (== GPC)
GCHUNKS = 32                   # edge chunks per input-stream DMA
S2S_STEPS = 3


def _f16(a):
    return np.ascontiguousarray(np.asarray(a, np.float32).astype(np.float16))


def _host_prep(inp):
    """Pure index/layout/dtype work: build per-core input maps."""
    x = np.asarray(inp["x"], np.float32)
    ea = np.asarray(inp["edge_attr"], np.float32)
    ei = np.asarray(inp["edge_index"])
    batch = np.asarray(inp["batch"]).astype(np.int64)
    src_all = np.asarray(ei[0], np.int64)
    dst_all = np.asarray(ei[1], np.int64)

    counts = np.bincount(batch, minlength=N_GRAPHS)
    assert counts.max() <= SLOT, f"graph larger than SLOT: {counts.max()}"
    starts = np.zeros(N_GRAPHS + 1, np.int64)
    np.cumsum(counts, out=starts[1:])

    dst_graph = batch[dst_all]
    dst_core = dst_graph // GPC

    # per-core per-graph edge counts; sort each core's graphs by edge count so
    # grid position j holds rank-j graphs on every core -> position-wise max
    # chunk counts track rank statistics (near-optimal padding, one program)
    epg_all = np.bincount(dst_graph, minlength=N_GRAPHS).reshape(N_CORES, GPC)
    pos_of_graph = np.empty(N_GRAPHS, np.int64)
    for k in range(N_CORES):
        perm = np.argsort(-epg_all[k], kind="stable")
        pos_of_graph[k * GPC + perm] = np.arange(GPC)
    epg_sorted = -np.sort(-epg_all, axis=1)
    chunks_pos = np.maximum(1, -(-epg_sorted.max(axis=0) // 128)).astype(np.int64)
    # variable slot window per position: max graph size at that rank, 32-aligned
    sizes_all = counts.reshape(N_CORES, GPC)
    wmax = np.zeros(GPC, np.int64)
    for k in range(N_CORES):
        perm = np.argsort(-epg_all[k], kind="stable")
        wmax = np.maximum(wmax, sizes_all[k][perm])
    windows = np.maximum(32, ((wmax + 31) // 32) * 32)
    # pack windows into 128-slot bins; PE out-base rule: w<=32 any 32-mult
    # base, w<=64 base {0,64}, else base 0
    order_w = np.argsort(-windows, kind="stable")
    binfill = []
    wbase = np.zeros(GPC, np.int64)
    wbin = np.zeros(GPC, np.int64)
    for j in order_w:
        w = int(windows[j])
        placed = False
        for bi, fill in enumerate(binfill):
            base = fill
            if base + w <= 128 and (w > 64 and base == 0 or
                                    32 < w <= 64 and base in (0, 64) or
                                    w <= 32):
                wbin[j] = bi; wbase[j] = base
                binfill[bi] = base + w
                placed = True
                break
        if not placed:
            wbin[j] = len(binfill); wbase[j] = 0
            binfill.append(w)
    NBIN = ((len(binfill) + 3) // 4) * 4
    wstart = wbin * 128 + wbase
    GRIDP = NBIN * 128
    gslot = wstart[pos_of_graph[batch]] + (np.arange(N_NODES) - starts[batch])
    dst_gslot = gslot[dst_all]
    pad4 = (-int(chunks_pos.sum())) % 4
    chunks_pos[-1] += pad4
    NECv = int(chunks_pos.sum())
    cstarts = np.zeros(GPC + 1, np.int64)
    np.cumsum(chunks_pos, out=cstarts[1:])
    EP = NECv * 128

    W0 = np.asarray(inp["W0"], np.float32); b0 = np.asarray(inp["b0"], np.float32)
    Wm1 = np.asarray(inp["Wm1"], np.float32); bm1 = np.asarray(inp["bm1"], np.float32)
    Wm2 = np.asarray(inp["Wm2"], np.float32); bm2 = np.asarray(inp["bm2"], np.float32)
    Wr = np.asarray(inp["Wr"], np.float32); br = np.asarray(inp["br"], np.float32)
    Wih = np.asarray(inp["Wih"], np.float32); Whh = np.asarray(inp["Whh"], np.float32)
    bl = np.asarray(inp["bl"], np.float32)
    W1 = np.asarray(inp["W1"], np.float32); b1 = np.asarray(inp["b1"], np.float32)
    W2 = np.asarray(inp["W2"], np.float32); b2 = np.asarray(inp["b2"], np.float32)

    W0c = _f16(np.concatenate([W0, b0[None, :]], 0))            # [26, 256]
    Wm1ec = _f16(np.concatenate([Wm1[D:], bm1[None, :]], 0))    # [15, 256]
    NECv_i = int(NECv)
    WBW = 2432 + NECv_i + NBIN
    wblob = np.zeros((128, WBW), np.float16)
    wblob[:26, 0:256] = W0c
    wblob[:15, 256:512] = Wm1ec
    wblob[:, 512:768] = _f16(Wm1[:128])
    wblob[:, 768:1024] = _f16(Wm1[128:D])
    wblob[:, 1024:1280] = _f16(Wm2[:128])
    wblob[:, 1280:1536] = _f16(Wm2[128:])
    wblob[:, 1536:1792] = _f16(Wr[:128])
    wblob[:, 1792:2048] = _f16(Wr[128:])
    wblob[:2, 2048:2304][:] = 0.0
    wblob[0, 2048:2304] = _f16(bm2)
    wblob[1, 2048:2304] = _f16(br)
    wblob[:, 2304:2432] = np.tile(np.arange(128, dtype=np.float16)[None, :],
                                  (128, 1))
    Wih_s = Wih.copy(); Wih_s[:D] *= 0.5                        # h state kept as 2h
    W1_s = W1.copy(); W1_s[:D] *= 0.5
    W1p = np.zeros((128, 4, 2, 128), np.float16)
    for kk in range(4):
        for m in range(2):
            W1p[:, kk, m, :] = _f16(W1_s[kk * 128:(kk + 1) * 128,
                                         m * 128:(m + 1) * 128])
    b1c = np.zeros((128, 2), np.float32)
    b1c[:, 0] = b1[:128]; b1c[:, 1] = b1[128:]
    W2s = np.zeros((128, 2), np.float16)
    W2s[:, 0] = _f16(W2[:128, 0]); W2s[:, 1] = _f16(W2[128:, 0])

    shared = dict(
        Wih=np.ascontiguousarray(_f16(Wih_s).reshape(4, 128, 1024).transpose(1, 0, 2)),
        Whh=np.ascontiguousarray(_f16(Whh * 0.5).reshape(2, 128, 1024).transpose(1, 0, 2)),
        blr=_f16(bl[None, :]),
        W1p=W1p, b1c=b1c, W2s=W2s, b2t=_f16(b2.reshape(1, 1)),
        onesr=np.ones((1, 128), np.float16),
        ident=np.eye(128, dtype=np.float16),
    )

    in_maps = []
    for k in range(N_CORES):
        g0 = k * GPC
        ns, ne = int(starts[g0]), int(starts[g0 + GPC])
        nodes = np.arange(ns, ne)
        gs = gslot[nodes]

        xTg = np.zeros((FIN + 1, GRIDP), np.float16)
        xTg[:FIN, gs] = _f16(x[nodes].T)
        xTg[FIN, :] = 1.0

        validg = np.zeros((128, NBIN), np.float16)
        for g in range(GPC):
            p = pos_of_graph[g0 + g]
            validg[wbase[p]:wbase[p] + counts[g0 + g], wbin[p]] = 1.0
        gr = batch[nodes] - g0
        gpos = pos_of_graph[batch[nodes]]
        gbinrel = gs - wbin[gpos] * 128
        GTp = np.zeros((64, GRIDP), np.float16)
        GTp[gr, gs] = 1.0
        Gp = np.zeros((128, NBIN * GPC), np.float16)
        Gp[gbinrel, wbin[gpos] * GPC + gr] = 1.0
        Gln = np.full((128, NBIN * GPC), -30.0, np.float16)
        Gln[gbinrel, wbin[gpos] * GPC + gr] = 0.0

        m = dst_core == k
        e_src = src_all[m]; e_slot = dst_gslot[m]; e_ea = ea[m]
        e_graph = pos_of_graph[dst_graph[m]]

        deg = np.zeros(GRIDP, np.float32)
        np.add.at(deg, e_slot, 1.0)
        degones = np.zeros((2, GRIDP), np.float16)
        degones[0] = deg.astype(np.float16); degones[1] = 1.0

        srcp = np.zeros(EP, np.int64)
        colp = np.full(EP, 255.0, np.float32)
        eap = np.zeros((EP, FE + 1), np.float16)
        order = np.argsort(e_graph, kind="stable")
        e_src, e_slot, e_ea = e_src[order], e_slot[order], e_ea[order]
        e_graph = e_graph[order]
        gstart = np.searchsorted(e_graph, np.arange(GPC + 1))
        for g in range(GPC):
            a, b = int(gstart[g]), int(gstart[g + 1])
            n_e = b - a
            assert n_e <= chunks_pos[g] * 128
            o = int(cstarts[g]) * 128
            srcp[o:o + n_e] = e_src[a:b]
            colp[o:o + n_e] = (e_slot[a:b] - wstart[g]).astype(np.float32)
            eap[o:o + n_e, :FE] = _f16(e_ea[a:b])
            eap[o:o + n_e, FE] = 1.0

        xgT = np.empty((FIN + 1, EP), np.float16)
        xgT[:FIN] = _f16(x[srcp].T)
        xgT[FIN] = 1.0
        dstcol = np.ascontiguousarray(colp.reshape(-1, 128).T.astype(np.float16))
        eaT = np.ascontiguousarray(
            eap.reshape(NECv, 128, FE + 1).transpose(2, 0, 1).reshape(FE + 1, EP))

        wb = wblob.copy()
        wb[:, 2432:2432 + NECv_i] = dstcol
        wb[:, 2432 + NECv_i:] = validg
        zoutc = np.zeros((128, NBIN, 257), np.float16)
        zoutc[:, :, 256] = validg
        im = dict(shared)
        im.update(xTg=xTg, wblob=wb, GTp=GTp, Gp=Gp, Gln=Gln, degones=degones,
                  xgT=xgT, eaT=eaT,
                  zout=np.ascontiguousarray(zoutc.reshape(128, -1)))
        in_maps.append(im)

    skip0 = bool(np.abs(bl).max() == 0.0)
    return in_maps, (tuple(int(c) for c in chunks_pos),
                     tuple(int(w) for w in windows),
                     tuple(int(w) for w in wstart), skip0)


def _build(nc, tc, chunks_pos, windows, wstart, skip0):
    """Emit one core's program (identical across cores; data differs).

    Edge phase is PE-bound; everything else is engine-balanced around it:
    - pair-batched edge MLP: one [128,512] PSUM tile per 2 chunks, a single
      relu per pair alternating ACT/DVE (halves relu instruction overhead)
    - scatter-matrix build (is_equal) on the otherwise-idle Pool engine
      (SBUF-only: GPSIMD has no PSUM port)
    - startup memsets on Pool so DVE is free from t=0
    - tp (h0 edge-source recompute) software-pipelined one pair ahead
    - agg/pout PSUM double-buffered so graph boundaries don't drain
    - per-graph tail (ags copy + out relu) alternates ACT/DVE by graph parity
    """
    NECv = sum(chunks_pos)
    EP = NECv * 128
    NBIN = (max(ws + w for ws, w in zip(wstart, windows)) + 127) // 128
    NBIN = ((NBIN + 3) // 4) * 4
    GRIDP = NBIN * 128
    NGG = GRIDP // 512
    # chunk index -> (graph, first-of-graph, last-of-graph)
    sched = []
    for g, cp in enumerate(chunks_pos):
        for jj in range(cp):
            sched.append((g, jj == 0, jj == cp - 1))

    def dram_in(name, shape, dt):
        return nc.dram_tensor(name, list(shape), dt, kind="ExternalInput")

    WBW = 2432 + NECv + NBIN
    xTg_d = dram_in("xTg", (FIN + 1, GRIDP), F16)
    xgT_d = dram_in("xgT", (FIN + 1, EP), F16)
    wblob_d = dram_in("wblob", (128, WBW), F16)
    Wih_d = dram_in("Wih", (128, 4, 1024), F16)
    Whh_d = dram_in("Whh", (128, 2, 1024), F16)
    blr_d = dram_in("blr", (1, 1024), F16)
    W1p_d = dram_in("W1p", (128, 4, 2, 128), F16)
    b1c_d = dram_in("b1c", (128, 2), F32)
    W2s_d = dram_in("W2s", (128, 2), F16)
    b2t_d = dram_in("b2t", (1, 1), F16)
    onesr_d = dram_in("onesr", (1, 128), F16)
    ident_d = dram_in("ident", (128, 128), F16)
    GTp_d = dram_in("GTp", (64, GRIDP), F16)
    zout_d = dram_in("zout", (128, NBIN * (D + 1)), F16)
    Gp_d = dram_in("Gp", (128, NBIN * GPC), F16)
    Gln_d = dram_in("Gln", (128, NBIN * GPC), F16)
    degones_d = dram_in("degones", (2, GRIDP), F16)
    eaT_d = dram_in("eaT", (FE + 1, EP), F16)

    y_d = nc.dram_tensor("y", [64, 1], F32, kind="ExternalOutput")

    def sb(name, shape, dt):
        return nc.alloc_sbuf_tensor(name, list(shape), dt).ap()

    s_wb = sb("s_wb", (128, WBW), F16)
    s_w0 = s_wb[0:FIN + 1, 0:256]
    s_wm1ec = s_wb[0:FE + 1, 256:512]
    s_wm1hi = s_wb[:, 512:768]
    s_wm1lo = s_wb[:, 768:1024]
    s_wm2hi = s_wb[:, 1024:1280]
    s_wm2lo = s_wb[:, 1280:1536]
    s_wrhi = s_wb[:, 1536:1792]
    s_wrlo = s_wb[:, 1792:2048]
    s_bmbr = s_wb[0:2, 2048:2304]
    s_iota = s_wb[:, 2304:2432]
    s_dstc16 = s_wb[:, 2432:2432 + NECv]
    s_valid = s_wb[:, 2432 + NECv:2432 + NECv + NBIN]
    s_xTg = sb("s_xTg", (FIN + 1, GRIDP), F16)
    s_dstcol = sb("s_dstcol", (128, NECv), F32)
    s_wih = sb("s_wih", (128, 4, 1024), F16)
    s_whh = sb("s_whh", (128, 2, 1024), F16)
    s_blr = sb("s_blr", (1, 1024), F16)
    s_w1 = sb("s_w1", (128, 4, 2, 128), F16)
    s_b1 = sb("s_b1", (128, 2), F32)
    s_w2 = sb("s_w2", (128, 2), F16)
    s_b2 = sb("s_b2", (1, 1), F16)
    s_onesr = sb("s_onesr", (1, 128), F16)
    s_ident = sb("s_ident", (128, 128), F16)
    s_GT = sb("s_GT", (64, GRIDP), F16)
    s_G = sb("s_G", (128, NBIN, GPC), F16)
    s_Gln = sb("s_Gln", (128, NBIN, GPC), F16)
    s_dego = sb("s_dego", (2, GRIDP), F16)
    s_h0g_hi = sb("s_h0g_hi", (128, GRIDP), F16)
    s_h0g_lo = sb("s_h0g_lo", (128, GRIDP), F16)
    s_out = sb("s_out", (128, NBIN, D + 1), F16)
    s_e = sb("s_e", (128, NBIN), F32)

    s_hT = [sb(f"s_hT{i}", (128, 64), F16) for i in range(2)]
    s_rT = [sb(f"s_rT{i}", (128, 64), F16) for i in range(2)]
    s_cu = sb("s_cu", (64, D), F32)
    s_hh16 = sb("s_hh16", (64, D), F16)
    s_y1 = [sb(f"s_y1_{i}", (128, 64), F16) for i in range(2)]
    s_yo = sb("s_yo", (64, 1), F32)

    dma = nc.sync.dma_start
    V, A, T = nc.vector, nc.scalar, nc.tensor
    GP = nc.gpsimd

    # inputs needed first: one packed blob + resident xTg
    dma(s_wb[:], wblob_d[:])
    dma(s_xTg[:], xTg_d[:])

    # s_out zero+valid fill comes in by DMA (idle engines; DVE stays free)
    dma(s_out[:], zout_d[:].rearrange("p (c g) -> p c g", g=D + 1))
    V.tensor_copy(s_dstcol[:], s_dstc16[:])
    for t_ in (*s_hT, *s_rT):
        V.memset(t_[:], 0.0)
    V.memset(s_cu[:], 0.0)

    # ============ P2: edge pipeline + segment sum + root update =============
    ndma = (NECv + GCHUNKS - 1) // GCHUNKS
    NP = NECv // 2  # chunk pairs (NECv is a multiple of 4)
    with tc.tile_pool(name="p2zg", bufs=3) as pzg, \
         tc.tile_pool(name="p2ea", bufs=3) as pea, \
         tc.tile_pool(name="p2t16", bufs=6) as pt16, \
         tc.tile_pool(name="p2m1", bufs=4) as pm1, \
         tc.tile_pool(name="p2S", bufs=10) as pS, \
         tc.tile_pool(name="p2ags", bufs=6) as pag, \
         tc.tile_pool(name="p2tp", bufs=2, space="PSUM") as ptp, \
         tc.tile_pool(name="p2eaw", bufs=3, space="PSUM") as peaw, \
         tc.tile_pool(name="p2agg", bufs=2, space="PSUM") as pagg, \
         tc.tile_pool(name="p2out", bufs=1, space="PSUM") as pout:

        strip_tiles = {}

        def get_strip(s):
            if s not in strip_tiles:
                n0 = s * GCHUNKS
                n1 = min(NECv, n0 + GCHUNKS)
                xgt = pzg.tile([FIN + 1, GCHUNKS * 128], F16, tag="xgt")
                dma(xgt[:, 0:(n1 - n0) * 128], xgT_d[:, n0 * 128:n1 * 128])
                eat = pea.tile([FE + 1, GCHUNKS * 128], F16, tag="eat")
                dma(eat[:, 0:(n1 - n0) * 128], eaT_d[:, n0 * 128:n1 * 128])
                strip_tiles[s] = (xgt, eat)
            return strip_tiles[s]

        def tp_stage(p):
            """h0 recompute for the 256 edge-sources of pair p -> f16 tiles."""
            c0 = 2 * p
            s = c0 // GCHUNKS
            off = (c0 - s * GCHUNKS) * 128
            xgt, _ = get_strip(s)
            tp_ = ptp.tile([128, 512], F32, tag="tp")
            th_, tl_ = tp_[:, 0:256], tp_[:, 256:512]
            T.matmul(th_, s_w0[:, 0:128], xgt[:, off:off + 256])
            T.matmul(tl_, s_w0[:, 128:256], xgt[:, off:off + 256])
            t16h = pt16.tile([128, 256], F16, tag="t16h")
            t16l = pt16.tile([128, 256], F16, tag="t16l")
            A.activation(t16h[:], th_, AF.Relu)
            V.tensor_relu(t16l[:], tl_)
            return t16h, t16l

        # prefetch + fill the tp pipeline
        get_strip(0)
        get_strip(1)
        get_strip(2)

        # Set2Set / readout weights: small, issue before the edge stream
        dma(s_G[:], Gp_d[:].rearrange("p (c g) -> p c g", g=GPC))
        dma(s_dego[:], degones_d[:])
        for sx, dx in [(s_wih, Wih_d), (s_whh, Whh_d), (s_blr, blr_d),
                       (s_w1, W1p_d), (s_b1, b1c_d), (s_w2, W2s_d),
                       (s_b2, b2t_d), (s_onesr, onesr_d),
                       (s_ident, ident_d), (s_GT, GTp_d)]:
            dma(sx[:], dx[:])
        dma(s_Gln[:], Gln_d[:].rearrange("p (c g) -> p c g", g=GPC))

        t16_of = {0: tp_stage(0)}
        for pp_ in (1, 2, 3, 4):
            if NP > pp_:
                t16_of[pp_] = tp_stage(pp_)

        # ---- P1 (grid h0T) tiles, interleaved into early pairs below.
        # Emission order: the tile containing graph-completion-rank-j's
        # window must land before that graph's (deferred) root update.
        NT1 = GRIDP // 256
        first_need = [NT1] * NT1
        for rank in range(GPC):
            t0 = wstart[rank] // 256
            t1 = (wstart[rank] + windows[rank] - 1) // 256
            for t_ in range(t0, t1 + 1):
                first_need[t_] = min(first_need[t_], rank)
        p1_order = sorted(range(NT1), key=lambda t_: first_need[t_])
        # One P1 tile every other pair, in first-need order.  (A "demand
        # schedule" that defers late-needed tiles into the mid-phase measured
        # WORSE: the early ramp-up pairs run PE at mid p-state, so ACT/DVE
        # absorb the P1 relus there essentially for free.)
        p1_sched = {2 * i + 1: [t1_] for i, t1_ in enumerate(p1_order)}

        def p1_tile(cg):
            sl = slice(cg * 256, (cg + 1) * 256)
            tp_ = ptp.tile([128, 512], F32, tag="tp")
            T.matmul(tp_[:, 0:256], s_w0[:, 0:128], s_xTg[:, sl])
            T.matmul(tp_[:, 256:512], s_w0[:, 128:256], s_xTg[:, sl])
            A.activation(s_h0g_hi[:, sl], tp_[:, 0:256], AF.Relu)
            V.tensor_relu(s_h0g_lo[:, sl], tp_[:, 256:512])

        m1_of = {}
        sc_of = {}      # pair -> list of (chunk idx, S tile)
        po_of = {}      # pair -> list of (graph, ags, gidx)
        agg = None
        ngraph = 0

        for p in range(NP + 5):
            if p < NP:
                c0 = 2 * p
                if c0 % GCHUNKS == 0:
                    s = c0 // GCHUNKS
                    if s + 2 < ndma:
                        get_strip(s + 2)
                # edge MLP1 for the pair: one [128,512] psum, 3 mm per chunk
                # (t16 produced two iterations ago -> no RAW stall on PE)
                s0 = c0 // GCHUNKS
                _, eat = get_strip(s0)
                ew = peaw.tile([128, 512], F32, tag="eaw")
                t16h, t16l = t16_of.pop(p)
                for h in range(2):
                    i = c0 + h
                    off = (i - s0 * GCHUNKS) * 128
                    dsl = slice(h * 256, h * 256 + 256)
                    hsl = slice(h * 128, h * 128 + 128)
                    T.matmul(ew[:, dsl], eat[:, off:off + 128], s_wm1ec[:],
                             start=True, stop=False)
                    T.matmul(ew[:, dsl], t16h[:, hsl], s_wm1hi[:],
                             start=False, stop=False)
                    T.matmul(ew[:, dsl], t16l[:, hsl], s_wm1lo[:],
                             start=False, stop=True)
                m1 = pm1.tile([128, 512], F16, tag="m1")
                A.activation(m1[:], ew[:], AF.Relu)
                m1_of[p] = m1
                if p + 5 < NP:
                    t16_of[p + 5] = tp_stage(p + 5)
                for t1_ in p1_sched.get(p, ()):
                    p1_tile(t1_)
                # scatter matrices for this pair (consumed two iterations on)
                sc = []
                for h in range(2):
                    i = c0 + h
                    w = windows[sched[i][0]]
                    S = pS.tile([128, 128], F16, tag="S")
                    V.tensor_scalar(S[:, 0:w], s_iota[:, 0:w],
                                    s_dstcol[:, i:i + 1], None,
                                    op0=ALU.is_equal)
                    sc.append(S)
                sc_of[p] = sc
            # ---- scatter for pair p-3 (m1 relu has long completed) ----
            q = p - 3
            if q in m1_of:
                m1q = m1_of.pop(q)
                scq = sc_of.pop(q)
                pos = []
                for h in range(2):
                    i = 2 * q + h
                    c, first, last = sched[i]
                    w = windows[c]
                    S = scq[h]
                    if first:
                        agg = pagg.tile([128, 2, 128], F32, tag="agg")
                    T.matmul(agg[:, 0, 0:w], m1q[:, h * 256:h * 256 + 128],
                             S[:, 0:w], start=first, stop=False)
                    T.matmul(agg[:, 1, 0:w], m1q[:, h * 256 + 128:h * 256 + 256],
                             S[:, 0:w], start=False, stop=last)
                    if last:
                        # drain agg psum right away (ACT/DVE by parity);
                        # one strided op for both halves
                        ags = pag.tile([128, 2, 128], F16, tag="ags")
                        V.tensor_copy(ags[:, :, 0:w], agg[:, :, 0:w])
                        pos.append((c, ags, ngraph))
                        ngraph += 1
                if pos:
                    po_of[q] = pos
            # ---- root update for graphs whose agg drained earlier ----
            q = p - 4
            if q in po_of:
                for c, ags, gi in po_of.pop(q):
                    w = windows[c]
                    po = pout.tile([128, D], F32, tag="po")
                    ws = wstart[c]
                    bn, ba = ws // 128, ws % 128
                    wsl = slice(ws, ws + w)
                    bsl = slice(ba, ba + w)
                    T.matmul(po[bsl, :], ags[:, 0, 0:w], s_wm2hi[:], start=True, stop=False)
                    T.matmul(po[bsl, :], ags[:, 1, 0:w], s_wm2lo[:], start=False, stop=False)
                    T.matmul(po[bsl, :], s_h0g_hi[:, wsl], s_wrhi[:], start=False, stop=False)
                    T.matmul(po[bsl, :], s_h0g_lo[:, wsl], s_wrlo[:], start=False, stop=False)
                    T.matmul(po[bsl, :], s_dego[:, wsl], s_bmbr[:], start=False, stop=True)
                    V.tensor_relu(s_out[bsl, bn, 0:D], po[bsl, :])

    # ============ P3: Set2Set (3 steps) + readout ===========================
    with tc.tile_pool(name="p3ps", bufs=2, space="PSUM") as pp3, \
         tc.tile_pool(name="p3p1", bufs=1, space="PSUM") as pq3, \
         tc.tile_pool(name="p3g", bufs=1, space="PSUM") as pg3, \
         tc.tile_pool(name="p3sb", bufs=2) as ps3:
        NBU0 = max(ws // 128 for ws in wstart) + 1
        if skip0:
            # Step-1 gates' h/bias terms depend only on weights and the
            # zero h state — emit them BEFORE the rw0 sweep so only the two
            # r-dependent matmuls per half sit on the serial entry path.
            # step-0 r = per-graph mean of out: one per-bin matmul sweep
            # (s_G is zero outside each graph's rows; col 256 sums valid)
            rw0p = pg3.tile([64, 257], F32, tag="rw")
            for b in range(NBU0):
                T.matmul(rw0p[:], s_G[:, b, :], s_out[:, b, :],
                         start=(b == 0), stop=(b == NBU0 - 1))
            rr = ps3.tile([64, 1], F32, tag="rr")
            V.reciprocal(rr[:], rw0p[:, 256:257])
            rf = ps3.tile([64, D], F16, tag="rf")
            V.tensor_scalar(rf[:], rw0p[:, 0:256], rr[:], None, op0=ALU.mult)
            for mth in range(2):
                ptr = pq3.tile([128, 64], F16, tag="ptr")
                T.transpose(ptr[:], rf[:, mth * 128:(mth + 1) * 128],
                            s_ident[0:64, 0:64])
                V.tensor_copy(s_rT[mth][:], ptr[:])
        for step in range(1 if skip0 else 0, S2S_STEPS):
            g0p = pg3.tile([64, 512], F32, tag="g0")
            g1p = pg3.tile([64, 512], F32, tag="g1")
            tif = ps3.tile([64, 2 * D], F32, tag="tif")
            tg = ps3.tile([64, D], F32, tag="tg")
            to = ps3.tile([64, D], F32, tag="to")
            a2 = ps3.tile([64, D], F32, tag="a2")
            bv = ps3.tile([64, D], F32, tag="bv")
            for half, gp in ((0, g0p), (1, g1p)):
                nsl = slice(half * 512, (half + 1) * 512)
                if not skip0:
                    # bl == 0 under skip0: the bias matmul would add zeros
                    T.matmul(gp[:], s_onesr[:, 0:64], s_blr[:, nsl],
                             start=True, stop=False)
                for kk in range(4):
                    lhs = (s_hT + s_rT)[kk]
                    T.matmul(gp[:], lhs[:], s_wih[:, kk, nsl],
                             start=(skip0 and kk == 0), stop=False)
                for kk in range(2):
                    T.matmul(gp[:], s_hT[kk][:], s_whh[:, kk, nsl],
                             start=False, stop=(kk == 1))
                if half == 0:
                    # i/f tanh + the c-gate product overlap g1p's matmuls
                    A.activation(tif[:], g0p[:, 0:512], AF.Tanh, scale=0.5)
                    V.scalar_tensor_tensor(a2[:], tif[:, D:2 * D], 1.0,
                                           s_cu[:], ALU.add, ALU.mult)
            A.activation(tg[:], g1p[:, 0:256], AF.Tanh)
            A.activation(to[:], g1p[:, 256:512], AF.Tanh, scale=0.5)
            V.scalar_tensor_tensor(bv[:], tif[:, 0:D], 1.0, tg[:],
                                   ALU.add, ALU.mult)
            V.scalar_tensor_tensor(s_cu[:], a2[:], 0.5, bv[:], ALU.mult, ALU.add)
            th = ps3.tile([64, D], F32, tag="th")
            A.activation(th[:], s_cu[:], AF.Tanh, scale=0.5)
            hh = ps3.tile([64, D], F32, tag="hh")
            V.scalar_tensor_tensor(hh[:], to[:], 1.0, th[:], ALU.add, ALU.mult)
            A.activation(s_hh16[:], hh[:], AF.Copy)
            # attention: per-chunk pipeline T->V->A->T with fused mask+exp.
            # hb needs only hh16, so it starts before the hT transposes,
            # which are deferred past the loop (only next step's gates and
            # the readout consume them).
            rw = pg3.tile([64, 257], F32, tag="rw")
            NBU = max(ws // 128 for ws in wstart) + 1
            for b in range(NBU):
                bsl = slice(b * 128, (b + 1) * 128)
                hb = pp3.tile([128, D], F32, tag="hb")
                T.matmul(hb[:], s_GT[:, bsl], s_hh16[:])
                scr = ps3.tile([128, D], F32, tag="scr")
                V.scalar_tensor_tensor(scr[:], s_out[:, b, 0:D], 0.5,
                                       hb[:], ALU.mult, ALU.mult,
                                       accum_out=s_e[:, b:b + 1])
                gw = ps3.tile([128, 64], F16, tag="gw")
                A.activation(gw[:], s_Gln[:, b, :], AF.Exp,
                             bias=s_e[:, b:b + 1])
                T.matmul(rw[:], gw[:], s_out[:, b, :],
                         start=(b == 0), stop=(b == NBU - 1))
            for mth in range(2):
                ptr = pq3.tile([128, 64], F16, tag="ptr")
                T.transpose(ptr[:], s_hh16[:, mth * 128:(mth + 1) * 128],
                            s_ident[0:64, 0:64])
                V.tensor_copy(s_hT[mth][:], ptr[:])
            rr = ps3.tile([64, 1], F32, tag="rr")
            V.reciprocal(rr[:], rw[:, 256:257])
            rf = ps3.tile([64, D], F16, tag="rf")
            V.tensor_scalar(rf[:], rw[:, 0:256], rr[:], None, op0=ALU.mult)
            for mth in range(2):
                ptr = pq3.tile([128, 64], F16, tag="ptr")
                T.transpose(ptr[:], rf[:, mth * 128:(mth + 1) * 128],
                            s_ident[0:64, 0:64])
                V.tensor_copy(s_rT[mth][:], ptr[:])
        # readout
        for mth in range(2):
            yp = pq3.tile([128, 64], F32, tag="yp")
            for kk in range(4):
                T.matmul(yp[:], s_w1[:, kk, mth, :], (s_hT + s_rT)[kk][:],
                         start=(kk == 0), stop=(kk == 3))
            A.activation(s_y1[mth][:], yp[:], AF.Relu, bias=s_b1[:, mth:mth + 1])
        ypo = pq3.tile([64, 1], F32, tag="ypo")
        T.matmul(ypo[:], s_y1[0][:], s_w2[:, 0:1], start=True, stop=False)
        T.matmul(ypo[:], s_y1[1][:], s_w2[:, 1:2], start=False, stop=False)
        T.matmul(ypo[:], s_onesr[:, 0:64], s_b2[:], start=False, stop=True)
        V.tensor_copy(s_yo[:], ypo[:])
        dma(y_d[:], s_yo[:])


_CACHE = {}
_EXEC_CACHE = {}


def _get_compiled(chunks_pos):
    key = chunks_pos
    if key not in _CACHE:
        chunks, windows, wst, skip0 = chunks_pos
        nc = bacc.Bacc("TRN2", target_bir_lowering=False, debug=False,
                       num_devices=N_CORES)
        with tile.TileContext(nc) as tc:
            _build(nc, tc, chunks, windows, wst, skip0)
        nc.compile()
        _CACHE[key] = nc
    return _CACHE[key]


def _get_exec(nc):
    """Build (once) a sharded PJRT executable for nc; repeat kernel() calls
    then skip jax re-tracing/compilation entirely."""
    if id(nc) in _EXEC_CACHE:
        return _EXEC_CACHE[id(nc)]
    import jax
    from jax.sharding import Mesh, PartitionSpec, NamedSharding
    from jax.experimental.shard_map import shard_map
    from concourse import bass2jax

    part_name = nc.partition_id_tensor.name if nc.partition_id_tensor else None
    in_names, out_names, out_avals, zero_outs = [], [], [], []
    for alloc in nc.m.functions[0].allocations:
        if not isinstance(alloc, mybir.MemoryLocationSet):
            continue
        name = alloc.memorylocations[0].name
        if alloc.kind == "ExternalInput" and name != part_name:
            in_names.append(name)
        elif alloc.kind == "ExternalOutput":
            out_names.append(name)
            sh = tuple(alloc.tensor_shape)
            dt = mybir.dt.np(alloc.dtype)
            out_avals.append(jax.core.ShapedArray(sh, dt))
            zero_outs.append(np.zeros(sh, dt))
    n_params = len(in_names)
    all_in = list(in_names) + list(out_names)
    if part_name:
        all_in.append(part_name)

    def _body(*args):
        operands = list(args)
        if part_name:
            operands.append(bass2jax.partition_id_tensor())
        return tuple(bass2jax._bass_exec_p.bind(
            *operands, out_avals=tuple(out_avals), in_names=tuple(all_in),
            out_names=tuple(out_names), lowering_input_output_aliases=(),
            sim_require_finite=True, sim_require_nnan=True, nc=nc))

    devices = jax.devices()[:N_CORES]
    mesh = Mesh(np.asarray(devices), ("core",))
    donate = tuple(range(n_params, n_params + len(out_names)))
    sharded = jax.jit(
        shard_map(_body, mesh=mesh,
                  in_specs=(PartitionSpec("core"),) * (n_params + len(out_names)),
                  out_specs=(PartitionSpec("core"),) * len(out_names)),
        donate_argnums=donate, keep_unused=True)
    shard = NamedSharding(mesh, PartitionSpec("core"))
    ctx = (sharded, in_names, out_names, zero_outs, shard, jax)
    _EXEC_CACHE[id(nc)] = ctx
    return ctx


def kernel(**inputs) -> np.ndarray:
    in_maps, chunks_pos = _host_prep(inputs)
    nc = _get_compiled(chunks_pos)
    sharded, in_names, out_names, zero_outs, shard, jax = _get_exec(nc)
    concat_in = [np.concatenate([np.asarray(in_maps[c][nm])
                                 for c in range(N_CORES)], 0)
                 for nm in in_names]
    dev_in = [jax.device_put(a, shard) for a in concat_in]
    zo = [jax.device_put(np.zeros((N_CORES * z.shape[0], *z.shape[1:]),
                                  z.dtype), shard)
          for z in zero_outs]
    outs = sharded(*dev_in, *zo)
    y = np.asarray(outs[out_names.index("y")]).reshape(-1)
    return y.astype(np.float32)



# revision 77
# speedup vs baseline: 1.1522x; 1.1522x over previous
"""Trainium2 Bass kernel for nn_DMPNN_Change_678604832935 (8-core SPMD DMPNN+Set2Set).

Sharding: each core owns 64 consecutive graphs (batch is sorted) plus all edges
whose dst node falls in those graphs — so segment_sum is core-local and no
collectives are needed.  The node-side h0 = relu(x@W0+b0) is recomputed per
edge-source from host-gathered x rows; since segment_sum is linear and sits
between the Wm2 matmul and the root update, m@Wm2 is folded to the node side
(16x fewer FLOPs), with deg(n)*bm2 as a rank-1 correction.

Perf structure (PE-throughput-bound edge phase; ~0.39 ms/core in TimelineSim
vs 0.49 for the pre-optimization version; edge-phase PE idle is ~5us):
- Per-core graphs are sorted by edge count so grid position j holds rank-j
  graphs on every core; per-position edge-chunk counts (max over cores) then
  track rank statistics -> near-minimal edge padding with ONE SPMD program.
- Per-position slot windows (32-aligned max graph size at that rank) are
  bin-packed into 128-slot grid bins (PE out-base rules: w<=64 at base 0/64,
  else base 0), shrinking the scatter stream and the attention loop (44 bins
  vs 64 graphs).
- The edge loop is a deep software pipeline over 2-chunk PAIRS, built
  around the fact that each engine executes its queue IN ORDER: at pair p the
  PE runs eaw(p) (one [128,512] psum, 3 matmuls/chunk), the h0 recompute for
  pair p+4, the scatter for pair p-3, root updates for graphs that finished
  at p-4 — so no PE instruction ever waits on a relu issued fewer than 3
  iterations earlier (lookahead depth was tuned by TimelineSim: 2->3 on the
  tp stage alone was worth 19us).  m1 relu is one [128,512] ACT op per pair; the
  per-graph agg drain + out-relu go to DVE (ACT~1010 / DVE~930 /
  PE~1100 ns per pair).  PSUM: tp ring 2 banks + eaw ring 3 + agg ring 2
  + po 1 = 8 banks exactly.
- P1 (grid h0) is interleaved one 256-col tile per other pair, emission
  ordered so each tile lands before the first root update that reads it.
- Startup: weights blob + xTg DMA first; s_out's zero+valid fill arrives as
  a DMA'd constant (no 12us DVE memset); 3-deep strip prefetch.
- Set2Set: with bl == 0 (checked at prep), step 0 collapses to a uniform
  average, computed at P3 entry as one accumulating matmul sweep over the
  44 bins (frees a P2 psum bank and 6.8us of P2-phase PE time).  The
  attention mask is folded into exp via a log-mask (gw = exp(Gln + e), f16
  flushes masked terms to 0), pipelining T->V->A->T per bin with no barrier.
  Softmax uses unnormalized exp (|e|<~8, validated); sigmoid is synthesized
  from tanh (one ACT table set); i/f gates share one [64,512] tanh.
- GPSIMD is unusable here: no PSUM port, and TensorScalarPtr fails the
  real ISA check on Pool (walrus rejects it; CoreSim does not).
"""

import os
import sys

for _p in ("/opt/trn_rl_repo", "/root/.axon_site/_ro/trn_rl_repo"):
    if os.path.isdir(_p) and _p not in sys.path:
        sys.path.append(_p)

import numpy as np

import concourse.bass as bass
import concourse.bacc as bacc
import concourse.mybir as mybir
import concourse.tile as tile
from concourse.bass_utils import run_bass_kernel_spmd

F16 = mybir.dt.float16
F32 = mybir.dt.float32
AF = mybir.ActivationFunctionType
ALU = mybir.AluOpType

N_NODES = 30000
FIN = 25
FE = 14
D = 256
N_GRAPHS = 512
N_CORES = 8
GPC = N_GRAPHS // N_CORES      # graphs per core
SLOT = 128                     # grid slots per graph
GRID = GPC * SLOT              # grid rows per core
NCH = GRID // 128              # grid chunks per core (== GPC)
GCHUNKS = 32                   # edge chunks per input-stream DMA
S2S_STEPS = 3


def _f16(a):
    return np.ascontiguousarray(np.asarray(a, np.float32).astype(np.float16))


def _host_prep(inp):
    """Pure index/layout/dtype work: build per-core input maps."""
    x = np.asarray(inp["x"], np.float32)
    ea = np.asarray(inp["edge_attr"], np.float32)
    ei = np.asarray(inp["edge_index"])
    batch = np.asarray(inp["batch"]).astype(np.int64)
    src_all = np.asarray(ei[0], np.int64)
    dst_all = np.asarray(ei[1], np.int64)

    counts = np.bincount(batch, minlength=N_GRAPHS)
    assert counts.max() <= SLOT, f"graph larger than SLOT: {counts.max()}"
    starts = np.zeros(N_GRAPHS + 1, np.int64)
    np.cumsum(counts, out=starts[1:])

    dst_graph = batch[dst_all]
    dst_core = dst_graph // GPC

    # per-core per-graph edge counts; sort each core's graphs by edge count so
    # grid position j holds rank-j graphs on every core -> position-wise max
    # chunk counts track rank statistics (near-optimal padding, one program)
    epg_all = np.bincount(dst_graph, minlength=N_GRAPHS).reshape(N_CORES, GPC)
    pos_of_graph = np.empty(N_GRAPHS, np.int64)
    for k in range(N_CORES):
        perm = np.argsort(-epg_all[k], kind="stable")
        pos_of_graph[k * GPC + perm] = np.arange(GPC)
    epg_sorted = -np.sort(-epg_all, axis=1)
    chunks_pos = np.maximum(1, -(-epg_sorted.max(axis=0) // 128)).astype(np.int64)
    # variable slot window per position: max graph size at that rank, 32-aligned
    sizes_all = counts.reshape(N_CORES, GPC)
    wmax = np.zeros(GPC, np.int64)
    for k in range(N_CORES):
        perm = np.argsort(-epg_all[k], kind="stable")
        wmax = np.maximum(wmax, sizes_all[k][perm])
    windows = np.maximum(32, ((wmax + 31) // 32) * 32)
    # pack windows into 128-slot bins; PE out-base rule: w<=32 any 32-mult
    # base, w<=64 base {0,64}, else base 0
    order_w = np.argsort(-windows, kind="stable")
    binfill = []
    wbase = np.zeros(GPC, np.int64)
    wbin = np.zeros(GPC, np.int64)
    for j in order_w:
        w = int(windows[j])
        placed = False
        for bi, fill in enumerate(binfill):
            base = fill
            if base + w <= 128 and (w > 64 and base == 0 or
                                    32 < w <= 64 and base in (0, 64) or
                                    w <= 32):
                wbin[j] = bi; wbase[j] = base
                binfill[bi] = base + w
                placed = True
                break
        if not placed:
            wbin[j] = len(binfill); wbase[j] = 0
            binfill.append(w)
    NBIN = ((len(binfill) + 3) // 4) * 4
    wstart = wbin * 128 + wbase
    GRIDP = NBIN * 128
    gslot = wstart[pos_of_graph[batch]] + (np.arange(N_NODES) - starts[batch])
    dst_gslot = gslot[dst_all]
    pad4 = (-int(chunks_pos.sum())) % 4
    chunks_pos[-1] += pad4
    NECv = int(chunks_pos.sum())
    cstarts = np.zeros(GPC + 1, np.int64)
    np.cumsum(chunks_pos, out=cstarts[1:])
    EP = NECv * 128

    W0 = np.asarray(inp["W0"], np.float32); b0 = np.asarray(inp["b0"], np.float32)
    Wm1 = np.asarray(inp["Wm1"], np.float32); bm1 = np.asarray(inp["bm1"], np.float32)
    Wm2 = np.asarray(inp["Wm2"], np.float32); bm2 = np.asarray(inp["bm2"], np.float32)
    Wr = np.asarray(inp["Wr"], np.float32); br = np.asarray(inp["br"], np.float32)
    Wih = np.asarray(inp["Wih"], np.float32); Whh = np.asarray(inp["Whh"], np.float32)
    bl = np.asarray(inp["bl"], np.float32)
    W1 = np.asarray(inp["W1"], np.float32); b1 = np.asarray(inp["b1"], np.float32)
    W2 = np.asarray(inp["W2"], np.float32); b2 = np.asarray(inp["b2"], np.float32)

    W0c = _f16(np.concatenate([W0, b0[None, :]], 0))            # [26, 256]
    Wm1ec = _f16(np.concatenate([Wm1[D:], bm1[None, :]], 0))    # [15, 256]
    NECv_i = int(NECv)
    WBW = 2432 + NECv_i + NBIN
    wblob = np.zeros((128, WBW), np.float16)
    wblob[:26, 0:256] = W0c
    wblob[:15, 256:512] = Wm1ec
    wblob[:, 512:768] = _f16(Wm1[:128])
    wblob[:, 768:1024] = _f16(Wm1[128:D])
    wblob[:, 1024:1280] = _f16(Wm2[:128])
    wblob[:, 1280:1536] = _f16(Wm2[128:])
    wblob[:, 1536:1792] = _f16(Wr[:128])
    wblob[:, 1792:2048] = _f16(Wr[128:])
    wblob[:2, 2048:2304][:] = 0.0
    wblob[0, 2048:2304] = _f16(bm2)
    wblob[1, 2048:2304] = _f16(br)
    wblob[:, 2304:2432] = np.tile(np.arange(128, dtype=np.float16)[None, :],
                                  (128, 1))
    Wih_s = Wih.copy(); Wih_s[:D] *= 0.5                        # h state kept as 2h
    W1_s = W1.copy(); W1_s[:D] *= 0.5
    W1p = np.zeros((128, 4, 2, 128), np.float16)
    for kk in range(4):
        for m in range(2):
            W1p[:, kk, m, :] = _f16(W1_s[kk * 128:(kk + 1) * 128,
                                         m * 128:(m + 1) * 128])
    b1c = np.zeros((128, 2), np.float32)
    b1c[:, 0] = b1[:128]; b1c[:, 1] = b1[128:]
    W2s = np.zeros((128, 2), np.float16)
    W2s[:, 0] = _f16(W2[:128, 0]); W2s[:, 1] = _f16(W2[128:, 0])

    shared = dict(
        Wih=np.ascontiguousarray(_f16(Wih_s).reshape(4, 128, 1024).transpose(1, 0, 2)),
        Whh=np.ascontiguousarray(_f16(Whh * 0.5).reshape(2, 128, 1024).transpose(1, 0, 2)),
        blr=_f16(bl[None, :]),
        W1p=W1p, b1c=b1c, W2s=W2s, b2t=_f16(b2.reshape(1, 1)),
        onesr=np.ones((1, 128), np.float16),
        ident=np.eye(128, dtype=np.float16),
    )

    in_maps = []
    for k in range(N_CORES):
        g0 = k * GPC
        ns, ne = int(starts[g0]), int(starts[g0 + GPC])
        nodes = np.arange(ns, ne)
        gs = gslot[nodes]

        xTg = np.zeros((FIN + 1, GRIDP), np.float16)
        xTg[:FIN, gs] = _f16(x[nodes].T)
        xTg[FIN, :] = 1.0

        validg = np.zeros((128, NBIN), np.float16)
        for g in range(GPC):
            p = pos_of_graph[g0 + g]
            validg[wbase[p]:wbase[p] + counts[g0 + g], wbin[p]] = 1.0
        gr = batch[nodes] - g0
        gpos = pos_of_graph[batch[nodes]]
        gbinrel = gs - wbin[gpos] * 128
        GTp = np.zeros((64, GRIDP), np.float16)
        GTp[gr, gs] = 1.0
        Gp = np.zeros((128, NBIN * GPC), np.float16)
        Gp[gbinrel, wbin[gpos] * GPC + gr] = 1.0
        Gln = np.full((128, NBIN * GPC), -30.0, np.float16)
        Gln[gbinrel, wbin[gpos] * GPC + gr] = 0.0

        m = dst_core == k
        e_src = src_all[m]; e_slot = dst_gslot[m]; e_ea = ea[m]
        e_graph = pos_of_graph[dst_graph[m]]

        deg = np.zeros(GRIDP, np.float32)
        np.add.at(deg, e_slot, 1.0)
        degones = np.zeros((2, GRIDP), np.float16)
        degones[0] = deg.astype(np.float16); degones[1] = 1.0

        srcp = np.zeros(EP, np.int64)
        colp = np.full(EP, 255.0, np.float32)
        eap = np.zeros((EP, FE + 1), np.float16)
        order = np.argsort(e_graph, kind="stable")
        e_src, e_slot, e_ea = e_src[order], e_slot[order], e_ea[order]
        e_graph = e_graph[order]
        gstart = np.searchsorted(e_graph, np.arange(GPC + 1))
        for g in range(GPC):
            a, b = int(gstart[g]), int(gstart[g + 1])
            n_e = b - a
            assert n_e <= chunks_pos[g] * 128
            o = int(cstarts[g]) * 128
            srcp[o:o + n_e] = e_src[a:b]
            colp[o:o + n_e] = (e_slot[a:b] - wstart[g]).astype(np.float32)
            eap[o:o + n_e, :FE] = _f16(e_ea[a:b])
            eap[o:o + n_e, FE] = 1.0

        xgT = np.empty((FIN + 1, EP), np.float16)
        xgT[:FIN] = _f16(x[srcp].T)
        xgT[FIN] = 1.0
        dstcol = np.ascontiguousarray(colp.reshape(-1, 128).T.astype(np.float16))
        eaT = np.ascontiguousarray(
            eap.reshape(NECv, 128, FE + 1).transpose(2, 0, 1).reshape(FE + 1, EP))

        wb = wblob.copy()
        wb[:, 2432:2432 + NECv_i] = dstcol
        wb[:, 2432 + NECv_i:] = validg
        zoutc = np.zeros((128, NBIN, 257), np.float16)
        zoutc[:, :, 256] = validg
        im = dict(shared)
        im.update(xTg=xTg, wblob=wb, GTp=GTp, Gp=Gp, Gln=Gln, degones=degones,
                  xgT=xgT, eaT=eaT,
                  zout=np.ascontiguousarray(zoutc.reshape(128, -1)))
        in_maps.append(im)

    skip0 = bool(np.abs(bl).max() == 0.0)
    return in_maps, (tuple(int(c) for c in chunks_pos),
                     tuple(int(w) for w in windows),
                     tuple(int(w) for w in wstart), skip0)


def _build(nc, tc, chunks_pos, windows, wstart, skip0):
    """Emit one core's program (identical across cores; data differs).

    Edge phase is PE-bound; everything else is engine-balanced around it:
    - pair-batched edge MLP: one [128,512] PSUM tile per 2 chunks, a single
      relu per pair alternating ACT/DVE (halves relu instruction overhead)
    - scatter-matrix build (is_equal) on the otherwise-idle Pool engine
      (SBUF-only: GPSIMD has no PSUM port)
    - startup memsets on Pool so DVE is free from t=0
    - tp (h0 edge-source recompute) software-pipelined one pair ahead
    - agg/pout PSUM double-buffered so graph boundaries don't drain
    - per-graph tail (ags copy + out relu) alternates ACT/DVE by graph parity
    """
    NECv = sum(chunks_pos)
    EP = NECv * 128
    NBIN = (max(ws + w for ws, w in zip(wstart, windows)) + 127) // 128
    NBIN = ((NBIN + 3) // 4) * 4
    GRIDP = NBIN * 128
    NGG = GRIDP // 512
    # chunk index -> (graph, first-of-graph, last-of-graph)
    sched = []
    for g, cp in enumerate(chunks_pos):
        for jj in range(cp):
            sched.append((g, jj == 0, jj == cp - 1))

    def dram_in(name, shape, dt):
        return nc.dram_tensor(name, list(shape), dt, kind="ExternalInput")

    WBW = 2432 + NECv + NBIN
    xTg_d = dram_in("xTg", (FIN + 1, GRIDP), F16)
    xgT_d = dram_in("xgT", (FIN + 1, EP), F16)
    wblob_d = dram_in("wblob", (128, WBW), F16)
    Wih_d = dram_in("Wih", (128, 4, 1024), F16)
    Whh_d = dram_in("Whh", (128, 2, 1024), F16)
    blr_d = dram_in("blr", (1, 1024), F16)
    W1p_d = dram_in("W1p", (128, 4, 2, 128), F16)
    b1c_d = dram_in("b1c", (128, 2), F32)
    W2s_d = dram_in("W2s", (128, 2), F16)
    b2t_d = dram_in("b2t", (1, 1), F16)
    onesr_d = dram_in("onesr", (1, 128), F16)
    ident_d = dram_in("ident", (128, 128), F16)
    GTp_d = dram_in("GTp", (64, GRIDP), F16)
    zout_d = dram_in("zout", (128, NBIN * (D + 1)), F16)
    Gp_d = dram_in("Gp", (128, NBIN * GPC), F16)
    Gln_d = dram_in("Gln", (128, NBIN * GPC), F16)
    degones_d = dram_in("degones", (2, GRIDP), F16)
    eaT_d = dram_in("eaT", (FE + 1, EP), F16)

    y_d = nc.dram_tensor("y", [64, 1], F32, kind="ExternalOutput")

    def sb(name, shape, dt):
        return nc.alloc_sbuf_tensor(name, list(shape), dt).ap()

    s_wb = sb("s_wb", (128, WBW), F16)
    s_w0 = s_wb[0:FIN + 1, 0:256]
    s_wm1ec = s_wb[0:FE + 1, 256:512]
    s_wm1hi = s_wb[:, 512:768]
    s_wm1lo = s_wb[:, 768:1024]
    s_wm2hi = s_wb[:, 1024:1280]
    s_wm2lo = s_wb[:, 1280:1536]
    s_wrhi = s_wb[:, 1536:1792]
    s_wrlo = s_wb[:, 1792:2048]
    s_bmbr = s_wb[0:2, 2048:2304]
    s_iota = s_wb[:, 2304:2432]
    s_dstc16 = s_wb[:, 2432:2432 + NECv]
    s_valid = s_wb[:, 2432 + NECv:2432 + NECv + NBIN]
    s_xTg = sb("s_xTg", (FIN + 1, GRIDP), F16)
    s_dstcol = sb("s_dstcol", (128, NECv), F32)
    s_wih = sb("s_wih", (128, 4, 1024), F16)
    s_whh = sb("s_whh", (128, 2, 1024), F16)
    s_blr = sb("s_blr", (1, 1024), F16)
    s_w1 = sb("s_w1", (128, 4, 2, 128), F16)
    s_b1 = sb("s_b1", (128, 2), F32)
    s_w2 = sb("s_w2", (128, 2), F16)
    s_b2 = sb("s_b2", (1, 1), F16)
    s_onesr = sb("s_onesr", (1, 128), F16)
    s_ident = sb("s_ident", (128, 128), F16)
    s_GT = sb("s_GT", (64, GRIDP), F16)
    s_G = sb("s_G", (128, NBIN, GPC), F16)
    s_Gln = sb("s_Gln", (128, NBIN, GPC), F16)
    s_dego = sb("s_dego", (2, GRIDP), F16)
    s_h0g_hi = sb("s_h0g_hi", (128, GRIDP), F16)
    s_h0g_lo = sb("s_h0g_lo", (128, GRIDP), F16)
    s_out = sb("s_out", (128, NBIN, D + 1), F16)
    s_e = sb("s_e", (128, NBIN), F32)

    s_hT = [sb(f"s_hT{i}", (128, 64), F16) for i in range(2)]
    s_rT = [sb(f"s_rT{i}", (128, 64), F16) for i in range(2)]
    s_cu = sb("s_cu", (64, D), F32)
    s_hh16 = sb("s_hh16", (64, D), F16)
    s_y1 = [sb(f"s_y1_{i}", (128, 64), F16) for i in range(2)]
    s_yo = sb("s_yo", (64, 1), F32)

    dma = nc.sync.dma_start
    V, A, T = nc.vector, nc.scalar, nc.tensor
    GP = nc.gpsimd

    # inputs needed first: one packed blob + resident xTg
    dma(s_wb[:], wblob_d[:])
    dma(s_xTg[:], xTg_d[:])

    # s_out zero+valid fill comes in by DMA (idle engines; DVE stays free)
    dma(s_out[:], zout_d[:].rearrange("p (c g) -> p c g", g=D + 1))
    V.tensor_copy(s_dstcol[:], s_dstc16[:])
    for t_ in (*s_hT, *s_rT):
        V.memset(t_[:], 0.0)
    V.memset(s_cu[:], 0.0)

    # ============ P2: edge pipeline + segment sum + root update =============
    ndma = (NECv + GCHUNKS - 1) // GCHUNKS
    NP = NECv // 2  # chunk pairs (NECv is a multiple of 4)
    with tc.tile_pool(name="p2zg", bufs=3) as pzg, \
         tc.tile_pool(name="p2ea", bufs=3) as pea, \
         tc.tile_pool(name="p2t16", bufs=6) as pt16, \
         tc.tile_pool(name="p2m1", bufs=4) as pm1, \
         tc.tile_pool(name="p2S", bufs=10) as pS, \
         tc.tile_pool(name="p2ags", bufs=6) as pag, \
         tc.tile_pool(name="p2tp", bufs=2, space="PSUM") as ptp, \
         tc.tile_pool(name="p2eaw", bufs=3, space="PSUM") as peaw, \
         tc.tile_pool(name="p2agg", bufs=2, space="PSUM") as pagg, \
         tc.tile_pool(name="p2out", bufs=1, space="PSUM") as pout:

        strip_tiles = {}

        def get_strip(s):
            if s not in strip_tiles:
                n0 = s * GCHUNKS
                n1 = min(NECv, n0 + GCHUNKS)
                xgt = pzg.tile([FIN + 1, GCHUNKS * 128], F16, tag="xgt")
                dma(xgt[:, 0:(n1 - n0) * 128], xgT_d[:, n0 * 128:n1 * 128])
                eat = pea.tile([FE + 1, GCHUNKS * 128], F16, tag="eat")
                dma(eat[:, 0:(n1 - n0) * 128], eaT_d[:, n0 * 128:n1 * 128])
                strip_tiles[s] = (xgt, eat)
            return strip_tiles[s]

        def tp_stage(p):
            """h0 recompute for the 256 edge-sources of pair p -> f16 tiles."""
            c0 = 2 * p
            s = c0 // GCHUNKS
            off = (c0 - s * GCHUNKS) * 128
            xgt, _ = get_strip(s)
            tp_ = ptp.tile([128, 512], F32, tag="tp")
            th_, tl_ = tp_[:, 0:256], tp_[:, 256:512]
            T.matmul(th_, s_w0[:, 0:128], xgt[:, off:off + 256])
            T.matmul(tl_, s_w0[:, 128:256], xgt[:, off:off + 256])
            t16h = pt16.tile([128, 256], F16, tag="t16h")
            t16l = pt16.tile([128, 256], F16, tag="t16l")
            A.activation(t16h[:], th_, AF.Relu)
            V.tensor_relu(t16l[:], tl_)
            return t16h, t16l

        # prefetch + fill the tp pipeline
        get_strip(0)
        get_strip(1)
        get_strip(2)

        # Set2Set / readout weights: small, issue before the edge stream
        dma(s_G[:], Gp_d[:].rearrange("p (c g) -> p c g", g=GPC))
        dma(s_dego[:], degones_d[:])
        for sx, dx in [(s_wih, Wih_d), (s_whh, Whh_d), (s_blr, blr_d),
                       (s_w1, W1p_d), (s_b1, b1c_d), (s_w2, W2s_d),
                       (s_b2, b2t_d), (s_onesr, onesr_d),
                       (s_ident, ident_d), (s_GT, GTp_d)]:
            dma(sx[:], dx[:])
        dma(s_Gln[:], Gln_d[:].rearrange("p (c g) -> p c g", g=GPC))

        t16_of = {0: tp_stage(0)}
        for pp_ in (1, 2, 3, 4):
            if NP > pp_:
                t16_of[pp_] = tp_stage(pp_)

        # ---- P1 (grid h0T) tiles, interleaved into early pairs below.
        # Emission order: the tile containing graph-completion-rank-j's
        # window must land before that graph's (deferred) root update.
        NT1 = GRIDP // 256
        first_need = [NT1] * NT1
        for rank in range(GPC):
            t0 = wstart[rank] // 256
            t1 = (wstart[rank] + windows[rank] - 1) // 256
            for t_ in range(t0, t1 + 1):
                first_need[t_] = min(first_need[t_], rank)
        p1_order = sorted(range(NT1), key=lambda t_: first_need[t_])
        # One P1 tile every other pair, in first-need order.  (A "demand
        # schedule" that defers late-needed tiles into the mid-phase measured
        # WORSE: the early ramp-up pairs run PE at mid p-state, so ACT/DVE
        # absorb the P1 relus there essentially for free.)
        p1_sched = {2 * i + 1: [t1_] for i, t1_ in enumerate(p1_order)}

        def p1_tile(cg):
            sl = slice(cg * 256, (cg + 1) * 256)
            tp_ = ptp.tile([128, 512], F32, tag="tp")
            T.matmul(tp_[:, 0:256], s_w0[:, 0:128], s_xTg[:, sl])
            T.matmul(tp_[:, 256:512], s_w0[:, 128:256], s_xTg[:, sl])
            A.activation(s_h0g_hi[:, sl], tp_[:, 0:256], AF.Relu)
            V.tensor_relu(s_h0g_lo[:, sl], tp_[:, 256:512])

        m1_of = {}
        sc_of = {}      # pair -> list of (chunk idx, S tile)
        po_of = {}      # pair -> list of (graph, ags, gidx)
        agg = None
        ngraph = 0

        for p in range(NP + 5):
            if p < NP:
                c0 = 2 * p
                if c0 % GCHUNKS == 0:
                    s = c0 // GCHUNKS
                    if s + 2 < ndma:
                        get_strip(s + 2)
                # edge MLP1 for the pair: one [128,512] psum, 3 mm per chunk
                # (t16 produced two iterations ago -> no RAW stall on PE)
                s0 = c0 // GCHUNKS
                _, eat = get_strip(s0)
                ew = peaw.tile([128, 512], F32, tag="eaw")
                t16h, t16l = t16_of.pop(p)
                for h in range(2):
                    i = c0 + h
                    off = (i - s0 * GCHUNKS) * 128
                    dsl = slice(h * 256, h * 256 + 256)
                    hsl = slice(h * 128, h * 128 + 128)
                    T.matmul(ew[:, dsl], eat[:, off:off + 128], s_wm1ec[:],
                             start=True, stop=False)
                    T.matmul(ew[:, dsl], t16h[:, hsl], s_wm1hi[:],
                             start=False, stop=False)
                    T.matmul(ew[:, dsl], t16l[:, hsl], s_wm1lo[:],
                             start=False, stop=True)
                m1 = pm1.tile([128, 512], F16, tag="m1")
                A.activation(m1[:], ew[:], AF.Relu)
                m1_of[p] = m1
                if p + 5 < NP:
                    t16_of[p + 5] = tp_stage(p + 5)
                for t1_ in p1_sched.get(p, ()):
                    p1_tile(t1_)
                # scatter matrices for this pair (consumed two iterations on)
                sc = []
                for h in range(2):
                    i = c0 + h
                    w = windows[sched[i][0]]
                    S = pS.tile([128, 128], F16, tag="S")
                    V.tensor_scalar(S[:, 0:w], s_iota[:, 0:w],
                                    s_dstcol[:, i:i + 1], None,
                                    op0=ALU.is_equal)
                    sc.append(S)
                sc_of[p] = sc
            # ---- scatter for pair p-3 (m1 relu has long completed) ----
            q = p - 3
            if q in m1_of:
                m1q = m1_of.pop(q)
                scq = sc_of.pop(q)
                pos = []
                for h in range(2):
                    i = 2 * q + h
                    c, first, last = sched[i]
                    w = windows[c]
                    S = scq[h]
                    if first:
                        agg = pagg.tile([128, 2, 128], F32, tag="agg")
                    T.matmul(agg[:, 0, 0:w], m1q[:, h * 256:h * 256 + 128],
                             S[:, 0:w], start=first, stop=False)
                    T.matmul(agg[:, 1, 0:w], m1q[:, h * 256 + 128:h * 256 + 256],
                             S[:, 0:w], start=False, stop=last)
                    if last:
                        # drain agg psum right away (ACT/DVE by parity);
                        # one strided op for both halves
                        ags = pag.tile([128, 2, 128], F16, tag="ags")
                        V.tensor_copy(ags[:, :, 0:w], agg[:, :, 0:w])
                        pos.append((c, ags, ngraph))
                        ngraph += 1
                if pos:
                    po_of[q] = pos
            # ---- root update for graphs whose agg drained earlier ----
            q = p - 4
            if q in po_of:
                for c, ags, gi in po_of.pop(q):
                    w = windows[c]
                    po = pout.tile([128, D], F32, tag="po")
                    ws = wstart[c]
                    bn, ba = ws // 128, ws % 128
                    wsl = slice(ws, ws + w)
                    bsl = slice(ba, ba + w)
                    T.matmul(po[bsl, :], ags[:, 0, 0:w], s_wm2hi[:], start=True, stop=False)
                    T.matmul(po[bsl, :], ags[:, 1, 0:w], s_wm2lo[:], start=False, stop=False)
                    T.matmul(po[bsl, :], s_h0g_hi[:, wsl], s_wrhi[:], start=False, stop=False)
                    T.matmul(po[bsl, :], s_h0g_lo[:, wsl], s_wrlo[:], start=False, stop=False)
                    T.matmul(po[bsl, :], s_dego[:, wsl], s_bmbr[:], start=False, stop=True)
                    V.tensor_relu(s_out[bsl, bn, 0:D], po[bsl, :])

    # ============ P3: Set2Set (3 steps) + readout ===========================
    with tc.tile_pool(name="p3ps", bufs=2, space="PSUM") as pp3, \
         tc.tile_pool(name="p3p1", bufs=1, space="PSUM") as pq3, \
         tc.tile_pool(name="p3g", bufs=1, space="PSUM") as pg3, \
         tc.tile_pool(name="p3sb", bufs=2) as ps3:
        NBU0 = max(ws // 128 for ws in wstart) + 1
        if skip0:
            # Step-1 gates' h/bias terms depend only on weights and the
            # zero h state — emit them BEFORE the rw0 sweep so only the two
            # r-dependent matmuls per half sit on the serial entry path.
            # step-0 r = per-graph mean of out: one per-bin matmul sweep
            # (s_G is zero outside each graph's rows; col 256 sums valid)
            rw0p = pg3.tile([64, 257], F32, tag="rw")
            for b in range(NBU0):
                T.matmul(rw0p[:], s_G[:, b, :], s_out[:, b, :],
                         start=(b == 0), stop=(b == NBU0 - 1))
            rr = ps3.tile([64, 1], F32, tag="rr")
            V.reciprocal(rr[:], rw0p[:, 256:257])
            rf = ps3.tile([64, D], F16, tag="rf")
            V.tensor_scalar(rf[:], rw0p[:, 0:256], rr[:], None, op0=ALU.mult)
            for mth in range(2):
                ptr = pq3.tile([128, 64], F16, tag="ptr")
                T.transpose(ptr[:], rf[:, mth * 128:(mth + 1) * 128],
                            s_ident[0:64, 0:64])
                V.tensor_copy(s_rT[mth][:], ptr[:])
        for step in range(1 if skip0 else 0, S2S_STEPS):
            g0p = pg3.tile([64, 512], F32, tag="g0")
            g1p = pg3.tile([64, 512], F32, tag="g1")
            tif = ps3.tile([64, 2 * D], F32, tag="tif")
            tg = ps3.tile([64, D], F32, tag="tg")
            to = ps3.tile([64, D], F32, tag="to")
            a2 = ps3.tile([64, D], F32, tag="a2")
            bv = ps3.tile([64, D], F32, tag="bv")
            for half, gp in ((0, g0p), (1, g1p)):
                nsl = slice(half * 512, (half + 1) * 512)
                if not skip0:
                    # bl == 0 under skip0: the bias matmul would add zeros
                    T.matmul(gp[:], s_onesr[:, 0:64], s_blr[:, nsl],
                             start=True, stop=False)
                for kk in range(4):
                    lhs = (s_hT + s_rT)[kk]
                    T.matmul(gp[:], lhs[:], s_wih[:, kk, nsl],
                             start=(skip0 and kk == 0), stop=False)
                for kk in range(2):
                    T.matmul(gp[:], s_hT[kk][:], s_whh[:, kk, nsl],
                             start=False, stop=(kk == 1))
                if half == 0:
                    # i/f tanh + the c-gate product overlap g1p's matmuls
                    A.activation(tif[:], g0p[:, 0:512], AF.Tanh, scale=0.5)
                    V.scalar_tensor_tensor(a2[:], tif[:, D:2 * D], 1.0,
                                           s_cu[:], ALU.add, ALU.mult)
            A.activation(tg[:], g1p[:, 0:256], AF.Tanh)
            A.activation(to[:], g1p[:, 256:512], AF.Tanh, scale=0.5)
            V.scalar_tensor_tensor(bv[:], tif[:, 0:D], 1.0, tg[:],
                                   ALU.add, ALU.mult)
            V.scalar_tensor_tensor(s_cu[:], a2[:], 0.5, bv[:], ALU.mult, ALU.add)
            th = ps3.tile([64, D], F32, tag="th")
            A.activation(th[:], s_cu[:], AF.Tanh, scale=0.5)
            hh = ps3.tile([64, D], F32, tag="hh")
            V.scalar_tensor_tensor(hh[:], to[:], 1.0, th[:], ALU.add, ALU.mult)
            A.activation(s_hh16[:], hh[:], AF.Copy)
            # attention: per-chunk pipeline T->V->A->T with fused mask+exp.
            # hb needs only hh16, so it starts before the hT transposes,
            # which are deferred past the loop (only next step's gates and
            # the readout consume them).
            rw = pg3.tile([64, 257], F32, tag="rw")
            NBU = max(ws // 128 for ws in wstart) + 1
            for b in range(NBU):
                bsl = slice(b * 128, (b + 1) * 128)
                hb = pp3.tile([128, D], F32, tag="hb")
                T.matmul(hb[:], s_GT[:, bsl], s_hh16[:])
                scr = ps3.tile([128, D], F32, tag="scr")
                V.scalar_tensor_tensor(scr[:], s_out[:, b, 0:D], 0.5,
                                       hb[:], ALU.mult, ALU.mult,
                                       accum_out=s_e[:, b:b + 1])
                gw = ps3.tile([128, 64], F16, tag="gw")
                A.activation(gw[:], s_Gln[:, b, :], AF.Exp,
                             bias=s_e[:, b:b + 1])
                T.matmul(rw[:], gw[:], s_out[:, b, :],
                         start=(b == 0), stop=(b == NBU - 1))
            for mth in range(2):
                ptr = pq3.tile([128, 64], F16, tag="ptr")
                T.transpose(ptr[:], s_hh16[:, mth * 128:(mth + 1) * 128],
                            s_ident[0:64, 0:64])
                V.tensor_copy(s_hT[mth][:], ptr[:])
            rr = ps3.tile([64, 1], F32, tag="rr")
            V.reciprocal(rr[:], rw[:, 256:257])
            rf = ps3.tile([64, D], F16, tag="rf")
            V.tensor_scalar(rf[:], rw[:, 0:256], rr[:], None, op0=ALU.mult)
            for mth in range(2):
                ptr = pq3.tile([128, 64], F16, tag="ptr")
                T.transpose(ptr[:], rf[:, mth * 128:(mth + 1) * 128],
                            s_ident[0:64, 0:64])
                V.tensor_copy(s_rT[mth][:], ptr[:])
        # readout
        for mth in range(2):
            yp = pq3.tile([128, 64], F32, tag="yp")
            for kk in range(4):
                T.matmul(yp[:], s_w1[:, kk, mth, :], (s_hT + s_rT)[kk][:],
                         start=(kk == 0), stop=(kk == 3))
            A.activation(s_y1[mth][:], yp[:], AF.Relu, bias=s_b1[:, mth:mth + 1])
        ypo = pq3.tile([64, 1], F32, tag="ypo")
        T.matmul(ypo[:], s_y1[0][:], s_w2[:, 0:1], start=True, stop=False)
        T.matmul(ypo[:], s_y1[1][:], s_w2[:, 1:2], start=False, stop=False)
        T.matmul(ypo[:], s_onesr[:, 0:64], s_b2[:], start=False, stop=True)
        V.tensor_copy(s_yo[:], ypo[:])
        dma(y_d[:], s_yo[:])


_CACHE = {}
_EXEC_CACHE = {}


def _get_compiled(chunks_pos):
    key = chunks_pos
    if key not in _CACHE:
        chunks, windows, wst, skip0 = chunks_pos
        nc = bacc.Bacc("TRN2", target_bir_lowering=False, debug=False,
                       num_devices=N_CORES)
        with tile.TileContext(nc) as tc:
            _build(nc, tc, chunks, windows, wst, skip0)
        nc.compile()
        _CACHE[key] = nc
    return _CACHE[key]


def _get_exec(nc):
    """Build (once) a sharded PJRT executable for nc; repeat kernel() calls
    then skip jax re-tracing/compilation entirely."""
    if id(nc) in _EXEC_CACHE:
        return _EXEC_CACHE[id(nc)]
    import jax
    from jax.sharding import Mesh, PartitionSpec, NamedSharding
    from jax.experimental.shard_map import shard_map
    from concourse import bass2jax

    part_name = nc.partition_id_tensor.name if nc.partition_id_tensor else None
    in_names, out_names, out_avals, zero_outs = [], [], [], []
    for alloc in nc.m.functions[0].allocations:
        if not isinstance(alloc, mybir.MemoryLocationSet):
            continue
        name = alloc.memorylocations[0].name
        if alloc.kind == "ExternalInput" and name != part_name:
            in_names.append(name)
        elif alloc.kind == "ExternalOutput":
            out_names.append(name)
            sh = tuple(alloc.tensor_shape)
            dt = mybir.dt.np(alloc.dtype)
            out_avals.append(jax.core.ShapedArray(sh, dt))
            zero_outs.append(np.zeros(sh, dt))
    n_params = len(in_names)
    all_in = list(in_names) + list(out_names)
    if part_name:
        all_in.append(part_name)

    def _body(*args):
        operands = list(args)
        if part_name:
            operands.append(bass2jax.partition_id_tensor())
        return tuple(bass2jax._bass_exec_p.bind(
            *operands, out_avals=tuple(out_avals), in_names=tuple(all_in),
            out_names=tuple(out_names), lowering_input_output_aliases=(),
            sim_require_finite=True, sim_require_nnan=True, nc=nc))

    devices = jax.devices()[:N_CORES]
    mesh = Mesh(np.asarray(devices), ("core",))
    donate = tuple(range(n_params, n_params + len(out_names)))
    sharded = jax.jit(
        shard_map(_body, mesh=mesh,
                  in_specs=(PartitionSpec("core"),) * (n_params + len(out_names)),
                  out_specs=(PartitionSpec("core"),) * len(out_names)),
        donate_argnums=donate, keep_unused=True)
    shard = NamedSharding(mesh, PartitionSpec("core"))
    ctx = (sharded, in_names, out_names, zero_outs, shard, jax)
    _EXEC_CACHE[id(nc)] = ctx
    return ctx


def kernel(**inputs) -> np.ndarray:
    in_maps, chunks_pos = _host_prep(inputs)
    nc = _get_compiled(chunks_pos)
    sharded, in_names, out_names, zero_outs, shard, jax = _get_exec(nc)
    concat_in = [np.concatenate([np.asarray(in_maps[c][nm])
                                 for c in range(N_CORES)], 0)
                 for nm in in_names]
    dev_in = [jax.device_put(a, shard) for a in concat_in]
    zo = [jax.device_put(np.zeros((N_CORES * z.shape[0], *z.shape[1:]),
                                  z.dtype), shard)
          for z in zero_outs]
    outs = sharded(*dev_in, *zo)
    y = np.asarray(outs[out_names.index("y")]).reshape(-1)
    return y.astype(np.float32)

